# revision 1
# baseline (speedup 1.0000x reference)
"""Causal multi-head attention block (B=8, T=1024, C=768, H=12) on 8 TRN2
NeuronCores, data-parallel over the batch dimension (one batch element per
core, no collectives).

Per-core math (reference, note k/q/v split order and no 1/sqrt(d) scale):
    qkv = x @ W_in + b_in ;  k, q, v = split(qkv, 3)
    y_h = softmax(causal(q_h @ k_h^T)) @ v_h
    out = concat_h(y_h) @ W_out + b_out

Layout strategy (everything feeds the PE array with zero transposes of
attention matrices):
  - xT [C, T] via PE-transpose of x.
  - kT/qT [768, T] = W_chunk.T @ xT   (features on partitions).
  - v natural [T, 768] (+ per-head all-ones column -> lhsT [keys, 65]).
  - scoresT [keys, queries] = kT_h.T-slices @ qT_h; softmax denominator
    comes out of the AV matmul's ones-column as row 64 of av psum.
  - yT accumulates [C, T], which is exactly the lhsT of the out-projection.
Causality: matmul N-ranges are restricted to queries >= key-block start;
only the 128x128 diagonal block needs an upper-triangular 0/1 mask mult.

All matmul operands are float32r (TF32-like, full PE rate at N>=256,
~2e-4 rel error measured on hw). PSUM accumulates in fp32.
"""
import sys

sys.path.insert(0, "/opt/trn_rl_repo")

import numpy as np

import concourse.bass as bass  # noqa: F401  (import keeps concourse init order sane)
import concourse.mybir as mybir
import concourse.tile as tile
from concourse import bacc
from concourse.masks import make_identity, make_upper_triangular

f32 = mybir.dt.float32
f32r = mybir.dt.float32r
ADD = mybir.AluOpType.add
EXP = mybir.ActivationFunctionType.Exp
COPY = mybir.ActivationFunctionType.Copy
IDENT = mybir.ActivationFunctionType.Identity

B, T, C, H, D = 8, 1024, 768, 12, 64
CK = C // 128  # 6 feature chunks
TK = T // 128  # 8 time chunks
N_CORES = 8


def _segs(vis0):
    """512-aligned matmul segments covering queries [vis0, T)."""
    if vis0 < 512:
        return [(vis0, 512), (512, T)]
    return [(vis0, T)]


def build_nc(reps=1):
    nc = bacc.Bacc(None)
    x_d = nc.dram_tensor("x", [T, C], f32r, kind="ExternalInput")
    Win_d = nc.dram_tensor("W_in", [C, 3 * C], f32r, kind="ExternalInput")
    bin_d = nc.dram_tensor("b_in", [3 * C], f32r, kind="ExternalInput")
    Wout_d = nc.dram_tensor("W_out", [C, C], f32r, kind="ExternalInput")
    bout_d = nc.dram_tensor("b_out", [C], f32r, kind="ExternalInput")
    y_d = nc.dram_tensor("y", [T, C], f32, kind="ExternalOutput")

    with tile.TileContext(nc) as tc:
      for _rep in range(reps):
        with tc.tile_pool(name="persist", bufs=1) as persist:
            kT = [persist.tile([128, T], f32r, name=f"kT{c}") for c in range(CK)]
            qT = [persist.tile([128, T], f32r, name=f"qT{c}") for c in range(CK)]
            vt = [persist.tile([128, H, D + 1], f32r, name=f"vt{t}") for t in range(TK)]
            yT = [persist.tile([128, T], f32r, name=f"yT{c}") for c in range(CK)]
            Wout_sb = [persist.tile([128, C], f32r, name=f"wo{c}") for c in range(CK)]
            bqk = persist.tile([128, 12], f32r, name="bqk")
            bv = persist.tile([128, C], f32, name="bv")
            bo = persist.tile([128, C], f32, name="bo")
            brow_v = persist.tile([1, C], f32r, name="brow_v")
            brow_o = persist.tile([1, C], f32r, name="brow_o")
            ones_r = persist.tile([128, 128], f32r, name="ones_r")
            idn_r = persist.tile([128, 128], f32r, name="idn_r")
            triT = persist.tile([128, 128], f32, name="triT")
            scratch = persist.tile([128, 128], f32, name="scratch")

            # ---- one-time setup (compute-only; DMAs issued after x below) ----
            make_identity(nc, scratch[:])  # memset 0 + affine_select
            nc.vector.tensor_copy(idn_r[:], scratch[:])
            nc.gpsimd.memset(scratch[:], 1.0)
            nc.vector.tensor_copy(ones_r[:], scratch[:])
            # triT[p, y] = 1 iff y >= p (key p visible to query y in diag block)
            make_upper_triangular(nc, triT[:], val=1.0, diag=True)

            # ---- phase 1: x -> xT, qkT, v ----
            with tc.tile_pool(name="xTp", bufs=1) as xTp:
                xT = [xTp.tile([128, T], f32r, name=f"xT{c}") for c in range(CK)]
                # 1a: load x, transpose 128x128 blocks on PE
                with (
                    tc.tile_pool(name="xpool", bufs=6) as xpool,
                    tc.tile_pool(name="tps", bufs=6, space="PSUM") as tps,
                ):
                    for ti in range(TK):
                        xc = xpool.tile([128, C], f32r, tag="xc")
                        nc.sync.dma_start(xc[:], x_d[ti * 128 : (ti + 1) * 128, :])
                        for c in range(CK):
                            ps = tps.tile([128, 128], f32r, tag="tp")
                            nc.tensor.transpose(
                                ps[:], xc[:, c * 128 : (c + 1) * 128], idn_r[:]
                            )
                            nc.vector.tensor_copy(
                                xT[c][:, ti * 128 : (ti + 1) * 128], ps[:]
                            )
                    # bias row loads (small) on the ACT HWDGE ring
                    nc.sync.dma_start(
                        bqk[:], bin_d[0 : 2 * C].rearrange("(o p) -> p o", p=128)
                    )
                    nc.sync.dma_start(
                        brow_v[:], bin_d[2 * C : 3 * C].rearrange("(a x) -> a x", a=1)
                    )
                    nc.sync.dma_start(
                        brow_o[:], bout_d[:].rearrange("(a x) -> a x", a=1)
                    )
                with tc.tile_pool(name="wv", bufs=1) as wvp:
                  with (
                    tc.tile_pool(name="wpool", bufs=12) as wpool,
                    tc.tile_pool(name="qkps", bufs=3, space="PSUM") as qkps,
                  ):
                    # 1b: kT/qT = W_chunk.T @ xT, bias fused into psum->sbuf copy
                    wv = []
                    bcasts_done = False
                    for g in range(4):  # groups of 3 feature chunks of k|q
                        wts = []
                        for c in range(CK):
                            wt = wpool.tile([128, 384], f32r, tag="wt")
                            nc.sync.dma_start(
                                wt[:],
                                Win_d[c * 128 : (c + 1) * 128, g * 384 : (g + 1) * 384],
                            )
                            wts.append(wt)
                        if g == 1:
                            for c in range(CK):
                                wvt = wvp.tile([128, C], f32r, name=f"wv{c}")
                                nc.sync.dma_start(
                                    wvt[:],
                                    Win_d[c * 128 : (c + 1) * 128, 2 * C : 3 * C],
                                )
                                wv.append(wvt)
                        if g == 3:
                            for c in range(CK):
                                nc.sync.dma_start(
                                    Wout_sb[c][:], Wout_d[c * 128 : (c + 1) * 128, :]
                                )
                        if g == 2 and not bcasts_done:
                            bcasts_done = True
                            for dst, row in ((bv, brow_v), (bo, brow_o)):
                                for s, w in ((0, 512), (512, 256)):
                                    ps = qkps.tile([128, 512], f32, tag="sps",
                                                   bufs=2, name="sps_t")
                                    nc.tensor.matmul(
                                        ps[:, :w], ones_r[0:1, :],
                                        row[:, s : s + w],
                                        start=True, stop=True,
                                    )
                                    nc.vector.tensor_copy(
                                        dst[:, s : s + w], ps[:, :w]
                                    )
                        for j3 in range(3):
                            jc = g * 3 + j3
                            dst = kT[jc] if jc < CK else qT[jc - CK]
                            ps = qkps.tile([128, T], f32, tag="qk")
                            for half in range(2):
                                sl = slice(half * 512, (half + 1) * 512)
                                for c in range(CK):
                                    nc.tensor.matmul(
                                        ps[:, sl],
                                        wts[c][:, j3 * 128 : (j3 + 1) * 128],
                                        xT[c][:, sl],
                                        start=(c == 0),
                                        stop=(c == CK - 1),
                                    )
                            nc.scalar.activation(
                                dst[:], ps[:], IDENT,
                                bias=bqk[:, jc : jc + 1].bitcast(f32),
                            )
                  # 1c: v natural [T, 768] + ones column, ti-serial with a
                  # 4-bank psum pool (leaves 4 banks untouched so phase 2's
                  # score pool needn't wait on the whole v-phase release)
                  with (
                    tc.tile_pool(name="vps", bufs=2, space="PSUM") as vps,
                  ):
                    for ti in range(TK):
                        ps = vps.tile([128, 1024], f32, tag="vps", name="vps_t")
                        tsl = slice(ti * 128, (ti + 1) * 128)
                        for c in range(CK):
                            for s, w in ((0, 512), (512, 256)):
                                nc.tensor.matmul(
                                    ps[:, s : s + w],
                                    xT[c][:, tsl],
                                    wv[c][:, s : s + w],
                                    start=(c == 0),
                                    stop=(c == CK - 1),
                                )
                        nc.vector.tensor_tensor(
                            vt[ti][:, :, 0:D],
                            ps[:, 0:C].rearrange("p (h d) -> p h d", h=H),
                            bv[:].rearrange("p (h d) -> p h d", h=H),
                            ADD,
                        )
                        nc.vector.tensor_copy(
                            vt[ti][:, :, D : D + 1],
                            ones_r[:, 0:H].rearrange("p (h o) -> p h o", o=1),
                        )

            # ---- phase 2: attention, one head at a time ----
            with (
                tc.tile_pool(name="scp", bufs=2, space="PSUM") as scp,
                tc.tile_pool(name="avp", bufs=2, space="PSUM") as avp,
                tc.tile_pool(name="etp", bufs=6) as etp,
            ):
                for h in range(H):
                    hc, hb = h // 2, 64 * (h % 2)
                    kTh = kT[hc][hb : hb + 64, :]
                    qTh = qT[hc][hb : hb + 64, :]
                    av = avp.tile([65, T], f32, tag="av")
                    # steps: kj 0..3 solo; then (4,7) and (5,6) packed into
                    # one sc tile / one exp each (their exps are overhead-
                    # dominated). av runs one step behind (software pipeline)
                    # so the PE always has the next sc ready for ACT.
                    steps = [(0,), (1,), (2, 7), (3, 6), (4, 5)]
                    done = []

                    def _packsegs(kjs):
                        """[(kj, base_off, q_start, q_end)] split at both the
                        query 512-boundaries and the sc-tile bank edges.
                        Solo steps keep absolute (query-aligned) offsets."""
                        out = []
                        base = kjs[0] * 128 if len(kjs) == 1 else 0
                        off = base
                        for kj in kjs:
                            vis0 = kj * 128
                            for s, e in _segs(vis0):
                                a = s
                                while a < e:
                                    o = off + (a - vis0)
                                    lim = min(e, a + (512 - o % 512))
                                    out.append((kj, off, a, lim))
                                    a = lim
                            off += T - vis0
                        return out, base, off

                    for si in range(len(steps) + 1):
                        if si < len(steps):
                            kjs = steps[si]
                            segs, base, tot = _packsegs(kjs)
                            sc = scp.tile([128, T], f32, tag="sc")
                            et = etp.tile([128, T], f32r, tag="et")
                            for kj, off, s, e in segs:
                                vis0 = kj * 128
                                o = off + (s - vis0)
                                nc.tensor.matmul(
                                    sc[:, o : o + (e - s)],
                                    kTh[:, vis0 : vis0 + 128],
                                    qTh[:, s:e],
                                    start=True,
                                    stop=True,
                                )
                            nc.scalar.activation(
                                et[:, base:tot], sc[:, base:tot], EXP
                            )
                            for kj, off, s, e in segs:
                                if s == kj * 128:  # leading diag block
                                    nc.vector.tensor_mul(
                                        et[:, off : off + 128],
                                        et[:, off : off + 128],
                                        triT[:],
                                    )
                            done.append((kjs, segs, et))
                        if si >= 1:
                            kjs, segs, et = done[si - 1]
                            for kj, off, s, e in segs:
                                o = off + (s - kj * 128)
                                nc.tensor.matmul(
                                    av[:, s:e],
                                    vt[kj][:, h, :],
                                    et[:, o : o + (e - s)],
                                    start=(kj == 0),
                                    stop=(e == 512 and kj == 3)
                                    or (e == T and kj == 5),
                                )
                    # softmax denominator: row 64 of av -> reciprocal ->
                    # gpsimd partition-broadcast (Pool engine is idle) -> mul
                    drow = etp.tile([1, T], f32, tag="drow", bufs=2, name="drow")
                    # note: partition-shifted write 64 -> 0 (hw-validated);
                    # gpsimd partition_broadcast reads physical partition 0
                    nc.vector.reciprocal(drow[0:1, :], av[64:65, :])
                    rbb = etp.tile([64, T], f32, tag="rb", bufs=2, name="rbb")
                    nc.gpsimd.partition_broadcast(rbb[:], drow[0:1, :])
                    nc.vector.tensor_mul(
                        yT[hc][hb : hb + 64, :],
                        av[0:64, :],
                        rbb[0:64, :],
                    )

            # ---- phase 3: out projection ----
            with (
                tc.tile_pool(name="opp", bufs=2, space="PSUM") as opp,
                tc.tile_pool(name="yp", bufs=3) as yp,
            ):
                for ti in range(TK):
                    tsl = slice(ti * 128, (ti + 1) * 128)
                    ps = opp.tile([128, 1024], f32, tag="op")
                    for s, w in ((0, 512), (512, 256)):
                        for c in range(CK):
                            nc.tensor.matmul(
                                ps[:, s : s + w],
                                yT[c][:, tsl],
                                Wout_sb[c][:, s : s + w],
                                start=(c == 0),
                                stop=(c == CK - 1),
                            )
                    ysb = yp.tile([128, C], f32, tag="y")
                    nc.vector.tensor_tensor(ysb[:], ps[:, 0:C], bo[:], ADD)
                    nc.sync.dma_start(y_d[tsl, :], ysb[:])

    nc.finalize()
    return nc


_NC_CACHE = []


def _get_nc():
    if not _NC_CACHE:
        _NC_CACHE.append(build_nc())
    return _NC_CACHE[0]


def _run(inputs, trace=False):
    from concourse.bass_utils import run_bass_kernel_spmd

    nc = _get_nc()
    x = np.ascontiguousarray(np.asarray(inputs["x"], dtype=np.float32))
    shared = {
        k: np.ascontiguousarray(np.asarray(inputs[k], dtype=np.float32))
        for k in ("W_in", "b_in", "W_out", "b_out")
    }
    in_maps = [{"x": x[b], **shared} for b in range(N_CORES)]
    res = run_bass_kernel_spmd(nc, in_maps, list(range(N_CORES)), trace=trace)
    out = np.stack([res.results[b]["y"] for b in range(N_CORES)], axis=0)
    return out, res


def kernel(**inputs):
    out, _ = _run(inputs, trace=False)
    return out


if __name__ == "__main__":
    rng = np.random.default_rng(0)
    inputs = {
        "x": rng.standard_normal((B, T, C), dtype=np.float32),
        "W_in": (rng.standard_normal((C, 3 * C), dtype=np.float32) * 0.02),
        "b_in": (rng.standard_normal((3 * C,), dtype=np.float32) * 0.02),
        "W_out": (rng.standard_normal((C, C), dtype=np.float32) * 0.02),
        "b_out": (rng.standard_normal((C,), dtype=np.float32) * 0.02),
    }
    out = kernel(**inputs)
    print("ok", out.shape, out.dtype)

